# revision 38
# baseline (speedup 1.0000x reference)
"""AttentionAugmentation2D kernel for 8 Trainium2 NeuronCores — v4.

Data-parallel over batch (B=8 -> 1 batch element per core).

Math (per batch, per head; H=W=32, L=H*W=1024, dh=32):
  logits[(x,y),(x',y')] = q.k + q.krw[y'-y+31] + q.krh[x'-x+31]
Both relative terms are folded into a single K=96 matmul:
  Q_aug = [qT; skew_w(q @ krw^T); skew_h(q @ krh^T)]   (96 x 1024 per head)
  K_aug = [kT; onehot32(y'); onehot32(x')]             (96 x 1024 per head)
logits are computed transposed (keys on partitions) so that exp(logitsT)
is directly the stationary operand of the attention@V matmul.

v4 structure (vs v3):
  - attention@V runs with the weights as the STATIONARY operand and V as
    the moving operand: out[q,d] accumulates over key chunks with only 33
    streamed columns per matmul (ap cost 33 vs 512).  The output lands
    q-on-partitions, which eliminates all 64 PE transposes and the at_sb
    evacuation copies of v3; the softmax denominator rides along as V's
    ones column and normalization is a tiny reciprocal+multiply per head.
  - q/k ship host-pre-transposed and are DMA'd straight into their
    QaugT/KaugT positions (no qkst staging tile, no partition-shift
    copies on DVE/Pool).
  - rel matmuls write two groups per PSUM tile and evacuate with a single
    strided copy per (dir, half, group-pair) on DVE/ACT.
  - exp of the 64 [128,1024] logit tiles is split between ACT (native
    Exp) and DVE ((e^s)^x tensor-tensor pow) to keep both lanes busy.
"""

import math
import numpy as np
import ml_dtypes

import concourse.bass as bass
import concourse.mybir as mybir
import concourse.tile as tile
from concourse import bacc
from concourse.bass_utils import run_bass_kernel_spmd

FP = mybir.dt.float32
BF = mybir.dt.bfloat16
AF = mybir.ActivationFunctionType

B = 8
H = W = 32
NH = 8
DH = 32          # per-head depth for q/k/v
L = H * W        # 1024 positions
SCALE = float(DH) ** -0.5
NT = L // 128    # 8 position tiles


def _build_onehot():
    # rows 0-31: onehot of y' = key % 32 ; rows 32-63: onehot of x' = key//32
    # pre-interleaved to the column layout col = pos*4 + hb, bf16 exact
    oh = np.zeros((64, L), dtype=np.float32)
    k = np.arange(L)
    oh[k % 32, k] = 1.0
    oh[32 + k // 32, k] = 1.0
    ohi = np.repeat(oh[:, :, None], 4, axis=2).reshape(64, 4 * L)
    return np.ascontiguousarray(ohi.astype(ml_dtypes.bfloat16))


def _build_nc():
    nc = bacc.Bacc(
        "TRN2",
        target_bir_lowering=False,
        debug=False,
        enable_asserts=True,
        num_devices=B,
    )
    # q/k depth rows host-transposed+interleaved. DMA cost is per-partition
    # bytes, so each 32-row group ships FOLDED onto 128 partitions
    # ([32 d, 4096] -> [4 fold x 32 d, 1024]); narrow engine copies unfold.
    # Group order: qh0, kh0, qh1, kh1 (deadline order).
    qki = nc.declare_dram_parameter("qki", [128, 4, 1024], BF, isOutput=False)
    krwh = nc.declare_dram_parameter("krwhT", [DH, 2 * (2 * W - 1)], BF, isOutput=False)
    oneh = nc.declare_dram_parameter("oneh", [64, 4 * L], BF, isOutput=False)
    xv = nc.declare_dram_parameter("xv", [L, NH * DH], BF, isOutput=False)
    out = nc.declare_dram_parameter("out", [L, NH * DH], FP, isOutput=True)

    def copy_on(eng, dst, src):
        if eng is nc.scalar:
            eng.copy(dst, src)
        else:
            eng.tensor_copy(dst, src)

    with tile.TileContext(nc) as tc:
        with (
            tc.tile_pool(name="const", bufs=1) as cp,
        ):
            krwh_sb = cp.tile([DH, 2 * (2 * W - 1)], BF)
            krw_sb = krwh_sb[:, 0 : 2 * W - 1]
            krh_sb = krwh_sb[:, 2 * W - 1 :]

            # interleaved column layout: col(half, pos, hb) =
            #   half*4096 + pos*4 + hb,  head h = half*4 + hb
            QaugT = cp.tile([96, 2, L, 4], BF)
            KaugT = cp.tile([96, 2, L, 4], BF)
            Vaug = cp.tile([128, NT, NH, DH + 2], BF)

            # deadline-ordered DMAs on parallel queues:
            #  ACT queue: krwh (rel matmuls need it first, tiny)
            #  SP queue: q half0 -> k half0 -> oneh half0 -> q/k/oneh half1
            #  Pool (swdge): V
            qkst = cp.tile([128, 4, 1024], BF, name="qkst")
            with tc.high_priority():
                nc.scalar.dma_start(out=krwh_sb, in_=krwh[:])
                for g in range(4):
                    nc.sync.dma_start(out=qkst[:, g, :], in_=qki[:, g, :])
                nc.sync.dma_start(
                    out=KaugT[32:96, 0].rearrange("p f h -> p (f h)"), in_=oneh[:]
                )
            with tc.tile_wait_until(0.004):
                nc.sync.dma_start(
                    out=KaugT[32:96, 1].rearrange("p f h -> p (f h)"), in_=oneh[:]
                )
            # V straight into its SBUF layout (leaves the ones column gap);
            # per-t pieces keep the DMA APs within 3 dims
            xvr = xv.rearrange("(t p) c -> p t c", p=128)
            with tc.tile_wait_until(0.006):
                for t in range(NT):
                    nc.sync.dma_start(
                        out=Vaug[:, t, :, 0:DH],
                        in_=xvr[:, t, :].rearrange("p (h d) -> p h d", d=DH),
                    )
            # ones column for the softmax denominator: engine memset, no DMA
            nc.vector.memset(
                Vaug[:, :, :, DH : DH + 1].rearrange("p t h o -> p (t h o)"), 1.0
            )

            # unfold copies: half-0 rows on DVE (fast, needed first),
            # half-1 on Pool (idle early)
            qf = QaugT[0:32].rearrange("p a (f c) h -> p a f (c h)", f=4)
            kf = KaugT[0:32].rearrange("p a (f c) h -> p a f (c h)", f=4)
            for f in range(4):
                nc.vector.tensor_copy(qf[:, 0, f], qkst[32 * f : 32 * f + 32, 0])
            for f in range(4):
                nc.vector.tensor_copy(kf[:, 0, f], qkst[32 * f : 32 * f + 32, 1])
            for f in range(4):
                nc.gpsimd.tensor_copy(qf[:, 1, f], qkst[32 * f : 32 * f + 32, 2])
            for f in range(4):
                nc.gpsimd.tensor_copy(kf[:, 1, f], qkst[32 * f : 32 * f + 32, 3])

            out_sb = cp.tile([128, NT, NH * DH], FP)
            # (e^SCALE)^logit == exp(SCALE*logit): lets the DVE compute the
            # softmax exp as a TensorTensor pow with a broadcast const base
            ebase = cp.tile([128, 1], FP)
            nc.vector.memset(ebase, math.exp(SCALE))
            # dummy exp pulls the ACT function-table load into the DMA wait
            junk = cp.tile([128, 1], FP)
            with tc.high_priority():
                nc.scalar.activation(junk, ebase, AF.Exp, scale=SCALE)

            # rel views (interleaved): free ordering per mm is (hb, x|y)
            q_i = QaugT[0:32]                                  # [32,2,L,4]
            qr = q_i.rearrange("p a (x y) h -> p a h x y", y=W)
            wd = QaugT[32:64].rearrange("p a (x y) h -> p a h x y", y=W)
            hd = QaugT[64:96].rearrange("p a (x y) h -> p a h x y", y=W)

            # rel groups are kt-aligned so the aug rows stream just in time:
            # QK for key chunk kt covers x' in [4kt,4kt+4), which needs the
            # w-rows for x-chunk kt//2 and the h-rows for v-group kt only.
            def rel_w_chunk(pool, half, xc, eng, tag="rp"):
                # all 32 y-windows restricted to 8 x columns, one PSUM tile
                rp = pool.tile(
                    [32, 32, 4, 8], FP, tag=tag, name=f"rw{half}_{xc}"
                )
                for v in range(W):
                    nc.tensor.matmul(
                        rp[:, v],
                        lhsT=krw_sb[:, 31 - v : 63 - v],
                        rhs=qr[:, half, :, 8 * xc : 8 * xc + 8, v],
                        start=True,
                        stop=True,
                    )
                dst = wd[:, half, :, 8 * xc : 8 * xc + 8, :].rearrange(
                    "p h x y -> p y h x"
                )
                copy_on(eng, dst, rp)

            def rel_h_group(pool, half, g, eng, tag="rp"):
                # one x-window group of 4 pre-skewed rel matmuls
                rp = pool.tile([32, 4, 4, 32], FP, tag=tag, name=f"rh{half}_{g}")
                for i in range(4):
                    v = 4 * g + i
                    nc.tensor.matmul(
                        rp[:, i],
                        lhsT=krh_sb[:, 31 - v : 63 - v],
                        rhs=qr[:, half, :, v, :],
                        start=True,
                        stop=True,
                    )
                dst = hd[:, half, :, 4 * g : 4 * g + 4, :].rearrange(
                    "p h i y -> p i h y"
                )
                copy_on(eng, dst, rp)

            def rel_h_pair(pool, half, gp, eng, tag="rp"):
                # two x-window groups in one PSUM tile (for half 1, where
                # streaming granularity doesn't matter), single evac
                rp = pool.tile(
                    [32, 2, 4, 4, 32], FP, tag=tag, name=f"rhp{half}_{gp}"
                )
                for gg in range(2):
                    for i in range(4):
                        v = 4 * (2 * gp + gg) + i
                        nc.tensor.matmul(
                            rp[:, gg, i],
                            lhsT=krh_sb[:, 31 - v : 63 - v],
                            rhs=qr[:, half, :, v, :],
                            start=True,
                            stop=True,
                        )
                dst = hd[:, half, :, 8 * gp : 8 * gp + 8, :].rearrange(
                    "p h (gg i) y -> p gg i h y", gg=2
                )
                copy_on(eng, dst, rp)

            # ---------------- rel half 0 (heads 0-3) ----------------------
            # issue order unblocks QK kt 0,1 first
            with tc.tile_pool(name="ps_rel", bufs=2, space="PSUM") as ps_rel:
                for xc in range(4):
                    rel_w_chunk(ps_rel, 0, xc, (nc.vector, nc.scalar)[xc % 2], tag="rw")
                    rel_h_group(ps_rel, 0, 2 * xc, nc.scalar, tag="rh")
                    rel_h_group(ps_rel, 0, 2 * xc + 1, nc.vector, tag="rh")

            # ---------------- attention over heads ------------------------
            with (
                tc.tile_pool(name="wt", bufs=3) as wtp,
                tc.tile_pool(name="stg", bufs=3) as stp,
                tc.tile_pool(name="ps_lt", bufs=3, space="PSUM") as ps_lt,
                tc.tile_pool(name="ps_av", bufs=2, space="PSUM") as ps_av,
            ):
                wts = {}
                avs = {}

                out_r = out.rearrange("(t p) c -> p t c", p=128)

                def finish_head(h):
                    av = avs.pop(h)
                    rcp = cp.tile([128, NT], FP, tag="rcp", name=f"rcp{h}")
                    nc.vector.reciprocal(rcp, av[:, :, DH])
                    rcp_b = bass.AP(
                        tensor=rcp.tensor,
                        offset=rcp.offset,
                        ap=[rcp.ap[0], rcp.ap[1], [0, DH]],
                    )
                    nc.vector.tensor_tensor(
                        out_sb[:, :, h * DH : (h + 1) * DH],
                        av[:, :, 0:DH],
                        rcp_b,
                        mybir.AluOpType.mult,
                    )
                    # stream this head's output columns out immediately
                    eng = (nc.sync, nc.gpsimd)[h % 2]
                    eng.dma_start(
                        out=out_r[:, :, h * DH : (h + 1) * DH],
                        in_=out_sb[:, :, h * DH : (h + 1) * DH],
                    )

                # rel half 1 pieces spread over heads 0-1 (heads 4-7 only
                # need them later); they ride the lt PSUM ring (same size)
                rel1 = [(xc, True) for xc in range(4)] + [(gp, False) for gp in range(4)]

                # exp engine split: ACT is a bit faster per tile than DVE
                # (1038 vs 1192 ns) and DVE carries the rel evacs + finish;
                # weight ACT heavier during the rel-loaded early heads.
                def exp_on_act_f(h, kt):
                    if h < 3:
                        return kt not in (2, 5)      # 6 of 8 on ACT
                    return kt % 2 == 0               # 4 of 8 on ACT

                def av_group(h, qt):
                    # one query tile's attention@V: 8 sequential accumulation
                    # matmuls (one pending PSUM group per bank at a time)
                    avp = avs[h]
                    for kt2 in range(NT):
                        nc.tensor.matmul(
                            avp[:, qt, 0 : DH + 1],
                            lhsT=wts[h][:, kt2, qt * 128 : (qt + 1) * 128],
                            rhs=Vaug[:, kt2, h, 0 : DH + 1],
                            start=(kt2 == 0),
                            stop=(kt2 == NT - 1),
                        )

                for h in range(NH):
                    wts[h] = wtp.tile(
                        [128, NT, L], BF, tag="wt", name=f"wt{h}"
                    )
                    ha, hb = h // 4, h % 4
                    # padded to one full 2KB PSUM bank so two heads' pending
                    # accumulation groups never share a zero region
                    avs[h] = ps_av.tile(
                        [128, NT, 64], FP, tag="av", name=f"av{h}"
                    )
                    for kt in range(NT):
                        if True:
                            lt = ps_lt.tile([128, L], FP, tag="lt")
                            for qc in range(2):
                                nc.tensor.matmul(
                                    lt[:, qc * 512 : (qc + 1) * 512],
                                    lhsT=KaugT[:, ha, kt * 128 : (kt + 1) * 128, hb],
                                    rhs=QaugT[:, ha, qc * 512 : (qc + 1) * 512, hb],
                                    start=True,
                                    stop=True,
                                )
                            if exp_on_act_f(h, kt):
                                nc.scalar.activation(
                                    wts[h][:, kt, :], lt, AF.Exp, scale=SCALE
                                )
                            else:
                                # pow is not ISA-legal on DVE: DVE evacuates
                                # the tile, Pool computes (e^SCALE)^x
                                stg = stp.tile([128, L], FP, tag="stg")
                                nc.vector.tensor_copy(stg, lt)
                                eb = bass.AP(
                                    tensor=ebase.tensor,
                                    offset=ebase.offset,
                                    ap=[ebase.ap[0], [0, L]],
                                )
                                nc.gpsimd.tensor_tensor(
                                    wts[h][:, kt, :], eb, stg,
                                    mybir.AluOpType.pow,
                                )
                        # attention@V for the previous head, one query tile
                        # per slot, interleaved with this head's QK
                        if h > 0:
                            av_group(h - 1, kt)
                        # rel half 1 during heads 1-2, riding the lt ring
                        # (same per-partition PSUM size); by head 1 the Pool
                        # unfold copies of the half-1 q rows are done
                        if 1 <= h < 3 and kt % 2 == 1:
                            slot = 4 * (h - 1) + kt // 2
                            g, wdir = rel1[slot]
                            eng = (nc.scalar, nc.vector)[slot % 2]
                            if wdir:
                                rel_w_chunk(ps_lt, 1, g, eng, tag="lt")
                            else:
                                rel_h_pair(ps_lt, 1, g, eng, tag="lt")
                    if h > 0:
                        del wts[h - 1]
                        finish_head(h - 1)

                # tail: last head's attention@V and finish
                for qt in range(NT):
                    av_group(NH - 1, qt)
                del wts[NH - 1]
                finish_head(NH - 1)
    nc.compile()
    return nc


_NC_CACHE = None


def _prep(inputs, key_rel_w, key_rel_h):
    xf32 = inputs.astype(np.float32).reshape(-1, L, 3 * NH * DH)
    nb = xf32.shape[0]
    # [g, hb, d, pos] -> rows g*32+d, cols pos*4+hb, then each group's
    # [32, 4096] folds onto 128 partitions as [4 fold x 32 d, 1024];
    # group order (qh0, kh0, qh1, kh1)
    qk4 = (
        xf32[:, :, 0:512].transpose(0, 2, 1).reshape(nb, 4, 4, DH, L)
        .transpose(0, 1, 3, 4, 2).reshape(nb, 4, DH, 4 * L)
    )[:, [0, 2, 1, 3]]
    # [b, g, d, (f c)] -> [b, (f d), g, c]
    qki = np.ascontiguousarray(
        qk4.reshape(nb, 4, DH, 4, 1024).transpose(0, 3, 2, 1, 4)
        .reshape(nb, 128, 4, 1024).astype(ml_dtypes.bfloat16)
    )
    xv = np.ascontiguousarray(
        xf32[:, :, 512:768].astype(ml_dtypes.bfloat16)
    )
    krwhT = np.ascontiguousarray(
        np.concatenate([key_rel_w, key_rel_h], axis=0)
        .astype(np.float32).T.astype(ml_dtypes.bfloat16)
    )
    return qki, xv, krwhT


def kernel(inputs: np.ndarray, key_rel_w: np.ndarray, key_rel_h: np.ndarray) -> np.ndarray:
    global _NC_CACHE
    qki, xv, krwhT = _prep(inputs, key_rel_w, key_rel_h)
    oneh = _build_onehot()

    if _NC_CACHE is None:
        _NC_CACHE = _build_nc()
    nc = _NC_CACHE

    in_maps = [
        {"qki": qki[b], "xv": xv[b], "krwhT": krwhT, "oneh": oneh}
        for b in range(B)
    ]
    res = run_bass_kernel_spmd(nc, in_maps, list(range(B)))
    o = np.stack([res.results[b]["out"] for b in range(B)], axis=0)
    return np.ascontiguousarray(o.reshape(B, H, W, NH * DH).astype(np.float32))


# revision 42
# speedup vs baseline: 1.0235x; 1.0235x over previous
"""AttentionAugmentation2D kernel for 8 Trainium2 NeuronCores — v4.

Data-parallel over batch (B=8 -> 1 batch element per core).

Math (per batch, per head; H=W=32, L=H*W=1024, dh=32):
  logits[(x,y),(x',y')] = q.k + q.krw[y'-y+31] + q.krh[x'-x+31]
Both relative terms are folded into a single K=96 matmul:
  Q_aug = [qT; skew_w(q @ krw^T); skew_h(q @ krh^T)]   (96 x 1024 per head)
  K_aug = [kT; onehot32(y'); onehot32(x')]             (96 x 1024 per head)
logits are computed transposed (keys on partitions) so that exp(logitsT)
is directly the stationary operand of the attention@V matmul.

v4 structure (vs v3):
  - attention@V runs with the weights as the STATIONARY operand and V as
    the moving operand: out[q,d] accumulates over key chunks with only 33
    streamed columns per matmul (ap cost 33 vs 512).  The output lands
    q-on-partitions, which eliminates all 64 PE transposes and the at_sb
    evacuation copies of v3; the softmax denominator rides along as V's
    ones column and normalization is a tiny reciprocal+multiply per head.
  - q/k ship host-pre-transposed and are DMA'd straight into their
    QaugT/KaugT positions (no qkst staging tile, no partition-shift
    copies on DVE/Pool).
  - rel matmuls write two groups per PSUM tile and evacuate with a single
    strided copy per (dir, half, group-pair) on DVE/ACT.
  - exp of the 64 [128,1024] logit tiles is split between ACT (native
    Exp) and DVE ((e^s)^x tensor-tensor pow) to keep both lanes busy.
"""

import math
import numpy as np
import ml_dtypes

import concourse.bass as bass
import concourse.mybir as mybir
import concourse.tile as tile
from concourse import bacc
from concourse.bass_utils import run_bass_kernel_spmd

FP = mybir.dt.float32
BF = mybir.dt.bfloat16
AF = mybir.ActivationFunctionType

B = 8
H = W = 32
NH = 8
DH = 32          # per-head depth for q/k/v
L = H * W        # 1024 positions
SCALE = float(DH) ** -0.5
NT = L // 128    # 8 position tiles


def _build_onehot():
    # rows 0-31: onehot of y' = key % 32 ; rows 32-63: onehot of x' = key//32
    # pre-interleaved to the column layout col = pos*4 + hb, bf16 exact
    oh = np.zeros((64, L), dtype=np.float32)
    k = np.arange(L)
    oh[k % 32, k] = 1.0
    oh[32 + k // 32, k] = 1.0
    ohi = np.repeat(oh[:, :, None], 4, axis=2).reshape(64, 4 * L)
    return np.ascontiguousarray(ohi.astype(ml_dtypes.bfloat16))


def _build_nc():
    nc = bacc.Bacc(
        "TRN2",
        target_bir_lowering=False,
        debug=False,
        enable_asserts=True,
        num_devices=B,
    )
    # q/k depth rows host-transposed+interleaved. DMA cost is per-partition
    # bytes, so each 32-row group ships FOLDED onto 128 partitions
    # ([32 d, 4096] -> [4 fold x 32 d, 1024]); narrow engine copies unfold.
    # Group order: qh0, kh0, qh1, kh1 (deadline order).
    qki = nc.declare_dram_parameter("qki", [128, 4, 1024], BF, isOutput=False)
    krwh = nc.declare_dram_parameter("krwhT", [DH, 2 * (2 * W - 1)], BF, isOutput=False)
    oneh = nc.declare_dram_parameter("oneh", [64, 4 * L], BF, isOutput=False)
    xv = nc.declare_dram_parameter("xv", [L, NH * DH], BF, isOutput=False)
    out = nc.declare_dram_parameter("out", [L, NH * DH], FP, isOutput=True)

    def copy_on(eng, dst, src):
        if eng is nc.scalar:
            eng.copy(dst, src)
        else:
            eng.tensor_copy(dst, src)

    with tile.TileContext(nc) as tc:
        with (
            tc.tile_pool(name="const", bufs=1) as cp,
        ):
            krwh_sb = cp.tile([DH, 2 * (2 * W - 1)], BF)
            krw_sb = krwh_sb[:, 0 : 2 * W - 1]
            krh_sb = krwh_sb[:, 2 * W - 1 :]

            # interleaved column layout: col(half, pos, hb) =
            #   half*4096 + pos*4 + hb,  head h = half*4 + hb
            QaugT = cp.tile([96, 2, L, 4], BF)
            KaugT = cp.tile([96, 2, L, 4], BF)
            Vaug = cp.tile([128, NT, NH, DH + 2], BF)

            # deadline-ordered DMAs on parallel queues:
            #  ACT queue: krwh (rel matmuls need it first, tiny)
            #  SP queue: q half0 -> k half0 -> oneh half0 -> q/k/oneh half1
            #  Pool (swdge): V
            qkst = cp.tile([128, 4, 1024], BF, name="qkst")
            with tc.high_priority():
                nc.scalar.dma_start(out=krwh_sb, in_=krwh[:])
                for g in range(4):
                    nc.sync.dma_start(out=qkst[:, g, :], in_=qki[:, g, :])
                nc.sync.dma_start(
                    out=KaugT[32:96, 0].rearrange("p f h -> p (f h)"), in_=oneh[:]
                )
            with tc.tile_wait_until(0.004):
                nc.sync.dma_start(
                    out=KaugT[32:96, 1].rearrange("p f h -> p (f h)"), in_=oneh[:]
                )
            # V straight into its SBUF layout (leaves the ones column gap);
            # per-t pieces keep the DMA APs within 3 dims
            xvr = xv.rearrange("(t p) c -> p t c", p=128)
            with tc.tile_wait_until(0.006):
                for t in range(NT):
                    nc.sync.dma_start(
                        out=Vaug[:, t, :, 0:DH],
                        in_=xvr[:, t, :].rearrange("p (h d) -> p h d", d=DH),
                    )
            # ones column for the softmax denominator: engine memset, no DMA
            nc.vector.memset(
                Vaug[:, :, :, DH : DH + 1].rearrange("p t h o -> p (t h o)"), 1.0
            )

            # unfold copies: half-0 rows on DVE (fast, needed first),
            # half-1 on Pool (idle early)
            qf = QaugT[0:32].rearrange("p a (f c) h -> p a f (c h)", f=4)
            kf = KaugT[0:32].rearrange("p a (f c) h -> p a f (c h)", f=4)
            for f in range(4):
                nc.vector.tensor_copy(qf[:, 0, f], qkst[32 * f : 32 * f + 32, 0])
            for f in range(4):
                nc.vector.tensor_copy(kf[:, 0, f], qkst[32 * f : 32 * f + 32, 1])
            for f in range(4):
                nc.gpsimd.tensor_copy(qf[:, 1, f], qkst[32 * f : 32 * f + 32, 2])
            for f in range(4):
                nc.gpsimd.tensor_copy(kf[:, 1, f], qkst[32 * f : 32 * f + 32, 3])

            out_sb = cp.tile([128, NT, NH * DH], FP)
            # (e^SCALE)^logit == exp(SCALE*logit): lets the DVE compute the
            # softmax exp as a TensorTensor pow with a broadcast const base
            ebase = cp.tile([128, 1], FP)
            nc.vector.memset(ebase, math.exp(SCALE))
            # dummy exp pulls the ACT function-table load into the DMA wait
            junk = cp.tile([128, 1], FP)
            with tc.high_priority():
                nc.scalar.activation(junk, ebase, AF.Exp, scale=SCALE)

            # rel views (interleaved): free ordering per mm is (hb, x|y)
            q_i = QaugT[0:32]                                  # [32,2,L,4]
            qr = q_i.rearrange("p a (x y) h -> p a h x y", y=W)
            wd = QaugT[32:64].rearrange("p a (x y) h -> p a h x y", y=W)
            hd = QaugT[64:96].rearrange("p a (x y) h -> p a h x y", y=W)

            # rel groups are kt-aligned so the aug rows stream just in time:
            # QK for key chunk kt covers x' in [4kt,4kt+4), which needs the
            # w-rows for x-chunk kt//2 and the h-rows for v-group kt only.
            def rel_w_chunk(pool, half, xc, eng, tag="rp"):
                # all 32 y-windows restricted to 8 x columns, one PSUM tile
                rp = pool.tile(
                    [32, 32, 4, 8], FP, tag=tag, name=f"rw{half}_{xc}"
                )
                for v in range(W):
                    nc.tensor.matmul(
                        rp[:, v],
                        lhsT=krw_sb[:, 31 - v : 63 - v],
                        rhs=qr[:, half, :, 8 * xc : 8 * xc + 8, v],
                        start=True,
                        stop=True,
                    )
                dst = wd[:, half, :, 8 * xc : 8 * xc + 8, :].rearrange(
                    "p h x y -> p y h x"
                )
                copy_on(eng, dst, rp)

            def rel_h_group(pool, half, g, eng, tag="rp"):
                # one x-window group of 4 pre-skewed rel matmuls
                rp = pool.tile([32, 4, 4, 32], FP, tag=tag, name=f"rh{half}_{g}")
                for i in range(4):
                    v = 4 * g + i
                    nc.tensor.matmul(
                        rp[:, i],
                        lhsT=krh_sb[:, 31 - v : 63 - v],
                        rhs=qr[:, half, :, v, :],
                        start=True,
                        stop=True,
                    )
                dst = hd[:, half, :, 4 * g : 4 * g + 4, :].rearrange(
                    "p h i y -> p i h y"
                )
                copy_on(eng, dst, rp)

            def rel_h_pair(pool, half, gp, eng, tag="rp"):
                # two x-window groups in one PSUM tile (for half 1, where
                # streaming granularity doesn't matter), single evac
                rp = pool.tile(
                    [32, 2, 4, 4, 32], FP, tag=tag, name=f"rhp{half}_{gp}"
                )
                for gg in range(2):
                    for i in range(4):
                        v = 4 * (2 * gp + gg) + i
                        nc.tensor.matmul(
                            rp[:, gg, i],
                            lhsT=krh_sb[:, 31 - v : 63 - v],
                            rhs=qr[:, half, :, v, :],
                            start=True,
                            stop=True,
                        )
                dst = hd[:, half, :, 8 * gp : 8 * gp + 8, :].rearrange(
                    "p h (gg i) y -> p gg i h y", gg=2
                )
                copy_on(eng, dst, rp)

            # ---------------- rel half 0 (heads 0-3) ----------------------
            # issue order unblocks QK kt 0,1 first
            with tc.tile_pool(name="ps_rel", bufs=2, space="PSUM") as ps_rel:
                for xc in range(4):
                    rel_w_chunk(ps_rel, 0, xc, (nc.vector, nc.scalar)[xc % 2], tag="rw")
                    rel_h_group(ps_rel, 0, 2 * xc, nc.scalar, tag="rh")
                    rel_h_group(ps_rel, 0, 2 * xc + 1, nc.vector, tag="rh")

            # ---------------- attention over heads ------------------------
            with (
                tc.tile_pool(name="wt", bufs=3) as wtp,
                tc.tile_pool(name="stg", bufs=3) as stp,
                tc.tile_pool(name="ps_lt", bufs=3, space="PSUM") as ps_lt,
                tc.tile_pool(name="ps_av", bufs=2, space="PSUM") as ps_av,
            ):
                wts = {}
                avs = {}

                out_r = out.rearrange("(t p) c -> p t c", p=128)

                def finish_head(h):
                    av = avs.pop(h)
                    rcp = cp.tile([128, NT], FP, tag="rcp", name=f"rcp{h}")
                    nc.vector.reciprocal(rcp, av[:, :, DH])
                    rcp_b = bass.AP(
                        tensor=rcp.tensor,
                        offset=rcp.offset,
                        ap=[rcp.ap[0], rcp.ap[1], [0, DH]],
                    )
                    nc.vector.tensor_tensor(
                        out_sb[:, :, h * DH : (h + 1) * DH],
                        av[:, :, 0:DH],
                        rcp_b,
                        mybir.AluOpType.mult,
                    )
                    # stream this head's output columns out immediately
                    eng = (nc.sync, nc.gpsimd)[h % 2]
                    eng.dma_start(
                        out=out_r[:, :, h * DH : (h + 1) * DH],
                        in_=out_sb[:, :, h * DH : (h + 1) * DH],
                    )

                # rel half 1 pieces spread over heads 0-1 (heads 4-7 only
                # need them later); they ride the lt PSUM ring (same size)
                rel1 = [(xc, True) for xc in range(4)] + [(gp, False) for gp in range(4)]

                # exp engine split: ACT is a bit faster per tile than DVE
                # (1038 vs 1192 ns) and DVE carries the rel evacs + finish;
                # weight ACT heavier during the rel-loaded early heads.
                def exp_on_act_f(h, kt):
                    # ~38/64 on ACT, interleaved (no long ACT runs, which
                    # would head-of-line-block the 3-deep lt ring)
                    if h % 4 == 3:
                        return kt % 2 == 0           # 4 of 8 on ACT
                    return kt not in (1, 4, 6)       # 5 of 8 on ACT

                def av_group(h, qt):
                    # one query tile's attention@V: 8 sequential accumulation
                    # matmuls (one pending PSUM group per bank at a time)
                    avp = avs[h]
                    for kt2 in range(NT):
                        nc.tensor.matmul(
                            avp[:, qt, 0 : DH + 1],
                            lhsT=wts[h][:, kt2, qt * 128 : (qt + 1) * 128],
                            rhs=Vaug[:, kt2, h, 0 : DH + 1],
                            start=(kt2 == 0),
                            stop=(kt2 == NT - 1),
                        )

                for h in range(NH):
                    wts[h] = wtp.tile(
                        [128, NT, L], BF, tag="wt", name=f"wt{h}"
                    )
                    ha, hb = h // 4, h % 4
                    # padded to one full 2KB PSUM bank so two heads' pending
                    # accumulation groups never share a zero region
                    avs[h] = ps_av.tile(
                        [128, NT, 64], FP, tag="av", name=f"av{h}"
                    )
                    for kt in range(NT):
                        if True:
                            lt = ps_lt.tile([128, L], FP, tag="lt")
                            for qc in range(2):
                                nc.tensor.matmul(
                                    lt[:, qc * 512 : (qc + 1) * 512],
                                    lhsT=KaugT[:, ha, kt * 128 : (kt + 1) * 128, hb],
                                    rhs=QaugT[:, ha, qc * 512 : (qc + 1) * 512, hb],
                                    start=True,
                                    stop=True,
                                )
                            if exp_on_act_f(h, kt):
                                nc.scalar.activation(
                                    wts[h][:, kt, :], lt, AF.Exp, scale=SCALE
                                )
                            else:
                                # pow is not ISA-legal on DVE: DVE evacuates
                                # the tile, Pool computes (e^SCALE)^x
                                stg = stp.tile([128, L], FP, tag="stg")
                                nc.vector.tensor_copy(stg, lt)
                                eb = bass.AP(
                                    tensor=ebase.tensor,
                                    offset=ebase.offset,
                                    ap=[ebase.ap[0], [0, L]],
                                )
                                nc.gpsimd.tensor_tensor(
                                    wts[h][:, kt, :], eb, stg,
                                    mybir.AluOpType.pow,
                                )
                        # attention@V for the previous head, one query tile
                        # per slot, interleaved with this head's QK
                        if h > 0:
                            av_group(h - 1, kt)
                        # rel half 1 spread over heads 1-3 (QK of head 4
                        # reads the full half-1 aug rows from kt 0, so all
                        # pieces must land by then), riding the lt ring
                        if 1 <= h < 4 and kt in (1, 4, 7):
                            slot = 3 * (h - 1) + (0 if kt == 1 else 1 if kt == 4 else 2)
                        else:
                            slot = 99
                        if slot < 8:
                            g, wdir = rel1[slot]
                            eng = (nc.scalar, nc.vector)[slot % 2]
                            if wdir:
                                rel_w_chunk(ps_lt, 1, g, eng, tag="lt")
                            else:
                                rel_h_pair(ps_lt, 1, g, eng, tag="lt")
                    if h > 0:
                        del wts[h - 1]
                        finish_head(h - 1)

                # tail: last head's attention@V and finish
                for qt in range(NT):
                    av_group(NH - 1, qt)
                del wts[NH - 1]
                finish_head(NH - 1)
    nc.compile()
    return nc


_NC_CACHE = None


def _prep(inputs, key_rel_w, key_rel_h):
    xf32 = inputs.astype(np.float32).reshape(-1, L, 3 * NH * DH)
    nb = xf32.shape[0]
    # [g, hb, d, pos] -> rows g*32+d, cols pos*4+hb, then each group's
    # [32, 4096] folds onto 128 partitions as [4 fold x 32 d, 1024];
    # group order (qh0, kh0, qh1, kh1)
    qk4 = (
        xf32[:, :, 0:512].transpose(0, 2, 1).reshape(nb, 4, 4, DH, L)
        .transpose(0, 1, 3, 4, 2).reshape(nb, 4, DH, 4 * L)
    )[:, [0, 2, 1, 3]]
    # [b, g, d, (f c)] -> [b, (f d), g, c]
    qki = np.ascontiguousarray(
        qk4.reshape(nb, 4, DH, 4, 1024).transpose(0, 3, 2, 1, 4)
        .reshape(nb, 128, 4, 1024).astype(ml_dtypes.bfloat16)
    )
    xv = np.ascontiguousarray(
        xf32[:, :, 512:768].astype(ml_dtypes.bfloat16)
    )
    krwhT = np.ascontiguousarray(
        np.concatenate([key_rel_w, key_rel_h], axis=0)
        .astype(np.float32).T.astype(ml_dtypes.bfloat16)
    )
    return qki, xv, krwhT


def kernel(inputs: np.ndarray, key_rel_w: np.ndarray, key_rel_h: np.ndarray) -> np.ndarray:
    global _NC_CACHE
    qki, xv, krwhT = _prep(inputs, key_rel_w, key_rel_h)
    oneh = _build_onehot()

    if _NC_CACHE is None:
        _NC_CACHE = _build_nc()
    nc = _NC_CACHE

    in_maps = [
        {"qki": qki[b], "xv": xv[b], "krwhT": krwhT, "oneh": oneh}
        for b in range(B)
    ]
    res = run_bass_kernel_spmd(nc, in_maps, list(range(B)))
    o = np.stack([res.results[b]["out"] for b in range(B)], axis=0)
    return np.ascontiguousarray(o.reshape(B, H, W, NH * DH).astype(np.float32))
